# revision 86
# baseline (speedup 1.0000x reference)
"""Windowed multi-head attention (DWAttention) Bass kernel for Trainium2.

Problem: x[B=2, n=64, N=256, C=384] -> per-window MHA (H=12, d=32) with fused
QKV projection + out_proj (no bias on out_proj, in_proj bias provided).

Strategy (8 NeuronCores, data-parallel over the B*n = 128 independent
windows -> 16 windows per core).

Cost-model facts driving the design (TRN2):
  * matmul time = out-free-size x cycles/row, independent of K and M.
  * bf16 runs 1 cycle/row always (fp32r needs free>=256 for full rate).
  * GpSimd (Pool) engine cannot touch PSUM; DVE/Act PSUM access costs a
    fixed ~125/185ns per instruction.
  * Act table load is 1.3us -> scalar engine runs ONLY Exp.
  * DMA XBAR transpose (16x128 tiles, 14ns/tile) does sbuf->sbuf bf16
    transposes for free (no PE/PSUM involvement).

Per window w (tokens N=256, channels C=384, 3 chan-tiles of 128):
  1. qk^T = W_qk @ x^T: 6 chan-major psum tiles [128,256] -> SBUF bf16 with
     per-partition bias add (DVE).  18 mms, free=256.
  2. v = x @ W_v^T: 2 token-major psum tiles [128,384] -> SBUF bf16 with
     broadcast bias add (DVE).  6 mms, free=384.
  3. Scores per head h: S^T[k,q] = k_h @ q_h^T (K=d=32): 24 mms free=256
     into [128,1024] psum (2 heads / 2 banks); exp via one Act op per
     2-head batch (scale=1/sqrt(d) fused) -> A^T bf16 in SBUF.
  4. attn@v token-major: out[q-tile, 32] per (head, q-tile, k-tile) with
     lhsT = A^T slice, rhs = v slice: 48 mms free=32; denominator via
     rhs=ones [128,1]: 48 mms free=1.  All accumulate in 2 psum banks
     [128, 512] = [12*32 o-cols | 12 den-cols | pad].
  5. recip(den) on DVE, raw o eviction psum->sbuf bf16 (DVE), per-head
     normalize SBUF->SBUF on GpSimd (24 tensor_scalar ops), then DMA XBAR
     transpose [128q,384c] -> oT [128c, 3, 128q] per q-tile.
  6. out = o @ W_o^T from oT: 6 mms free=384 -> psum, DVE evict, DMA out.

The 16 windows are software-pipelined: slot s emits proj(w=s),
attention(w=s-1), output(w=s-3) interleaved so PE never starves and PSUM
fits exactly in 8 banks: scores 2x2 + attn-out 2x1 + proj/fin 2x1.
"""

import numpy as np
from contextlib import ExitStack

import ml_dtypes

import concourse.bass as bass
import concourse.masks as masks
import concourse.mybir as mybir
import concourse.tile as tile
from concourse import bacc
from concourse.bass_utils import run_bass_kernel_spmd

# Problem constants (hardcoded per contract).
B, NWIN, N, C = 2, 64, 256, 384
H, D = 12, 32
SCALE = float(D) ** -0.5
NCORES = 8
WPC = (B * NWIN) // NCORES  # windows per core = 16
CT = C // 128               # channel tiles = 3
TT = N // 128               # token (q/k) tiles = 2
NB = H // 2                 # exp batches of 2 heads = 6

F32 = mybir.dt.float32
BF16 = mybir.dt.bfloat16
ADD = mybir.AluOpType.add
MULT = mybir.AluOpType.mult
EXP = mybir.ActivationFunctionType.Exp


def build_program(wpc=WPC, reps=0):
    """reps>0 wraps the whole per-core body in a hardware loop executing it
    reps times - used only for wall-clock HW timing (outputs unchanged)."""
    nc = bacc.Bacc()

    xt_h = nc.dram_tensor("xt", [wpc // 2, 128, CT, 2 * N], BF16, kind="ExternalInput")
    wqk_h = nc.dram_tensor("wqkt", [128, CT, 2 * C], BF16, kind="ExternalInput")
    wv_h = nc.dram_tensor("wvt", [128, CT, C], BF16, kind="ExternalInput")
    wo_h = nc.dram_tensor("wot", [128, CT, C], BF16, kind="ExternalInput")
    bqk_h = nc.dram_tensor("bqkt", [128, 2 * CT], F32, kind="ExternalInput")
    bvb_h = nc.dram_tensor("bvb", [128, C], F32, kind="ExternalInput")
    out_h = nc.dram_tensor("out", [wpc, TT, 128, C], F32, kind="ExternalOutput")

    with ExitStack() as ctx:
        tc = ctx.enter_context(tile.TileContext(nc))
        wpool = ctx.enter_context(tc.tile_pool(name="wpool", bufs=1))
        xpool = ctx.enter_context(tc.tile_pool(name="xpool", bufs=4))
        qkpool = ctx.enter_context(tc.tile_pool(name="qkpool", bufs=3))
        vpool = ctx.enter_context(tc.tile_pool(name="vpool", bufs=3))
        apool = ctx.enter_context(tc.tile_pool(name="apool", bufs=6))
        opool = ctx.enter_context(tc.tile_pool(name="opool", bufs=6))
        o2pool = ctx.enter_context(tc.tile_pool(name="o2pool", bufs=6))
        otpool = ctx.enter_context(tc.tile_pool(name="otpool", bufs=4))
        fpool = ctx.enter_context(tc.tile_pool(name="fpool", bufs=4))
        rpool = ctx.enter_context(tc.tile_pool(name="rpool", bufs=6))
        # PSUM: scores 2bufs x 2banks + attn-out 2bufs x 1bank
        #       + proj/fin 2bufs x 1bank = 8 banks exactly.
        sc_ps = ctx.enter_context(tc.tile_pool(name="sc_ps", bufs=2, space="PSUM"))
        o_ps = ctx.enter_context(tc.tile_pool(name="o_ps", bufs=2, space="PSUM"))
        pj_ps = ctx.enter_context(tc.tile_pool(name="pj_ps", bufs=2, space="PSUM"))

        st = {}  # per-window pipeline state

        def dma_x(p):
            """Load x^T for window pair (2p, 2p+1) in one tile: the QK
            projection batches both windows into free=512 matmuls."""
            xt = xpool.tile([128, CT, 2 * N], BF16, name="xt_sb")
            nc.sync.dma_start(out=xt, in_=xt_h.ap()[p])
            st[2 * p] = {"xt": xt, "par": 0}
            st[2 * p + 1] = {"xt": xt, "par": 1}

        # ---- one-time constants (ordered so the first proj's inputs land
        # first: x0 + wqk per c-tile + qk bias, then the rest; weight DMAs
        # ride the Activation engine's DGE queue, in parallel with SP's) ----
        xt0 = xpool.tile([128, CT, 2 * N], BF16, name="xt_sb")
        wqk_sb = wpool.tile([128, CT, 2 * C], BF16, name="wqk_sb")
        for c in range(CT):
            nc.sync.dma_start(out=wqk_sb[:, c, :], in_=wqk_h.ap()[:, c, :])
            nc.sync.dma_start(out=xt0[:, c, :], in_=xt_h.ap()[0, :, c, :])
        st[0] = {"xt": xt0, "par": 0}
        st[1] = {"xt": xt0, "par": 1}
        bqk_sb = wpool.tile([128, 2 * CT], F32, name="bqk_sb")
        nc.sync.dma_start(out=bqk_sb, in_=bqk_h.ap())
        wv_sb = wpool.tile([128, CT, C], BF16, name="wv_sb")
        nc.sync.dma_start(out=wv_sb, in_=wv_h.ap())
        bvb_sb = wpool.tile([128, C], F32, name="bvb_sb")
        nc.sync.dma_start(out=bvb_sb, in_=bvb_h.ap())
        wo_sb = wpool.tile([128, CT, C], BF16, name="wo_sb")
        nc.sync.dma_start(out=wo_sb, in_=wo_h.ap())
        ones_sb = wpool.tile([128, 1], BF16, name="ones_sb")
        nc.vector.memset(ones_sb, 1.0)
        ident_sb = wpool.tile([128, 128], BF16, name="ident_sb")
        masks.make_identity(nc, ident_sb)

        def proj_qk_chunk(w, j0):
            """Project q/k chan-tiles (j0, j0+CT) for windows (w, w+1) at
            once: the rhs spans the x^T pair tile, free=512 per matmul."""
            s = st[w]
            if "qk" not in s:
                s["qk"] = qkpool.tile([128, 2 * CT, 2 * N], BF16, name="qk_sb")
                st[w + 1]["qk"] = s["qk"]
            for j in (j0, j0 + CT):
                ps = pj_ps.tile([128, 2 * N], F32, tag="pj", name="ps_qk")
                for c in range(CT):
                    nc.tensor.matmul(
                        ps,
                        wqk_sb[:, c, 128 * j:128 * (j + 1)],
                        s["xt"][:, c, :],
                        start=(c == 0), stop=(c == CT - 1),
                    )
                nc.vector.tensor_scalar(
                    out=s["qk"][:, j, :], in0=ps,
                    scalar1=bqk_sb[:, j:j + 1], scalar2=None, op0=ADD,
                )

        def proj_v(w):
            s = st[w]
            v = vpool.tile([128, TT, C], BF16, name="v_sb")
            s["v"] = v
            for m in range(TT):
                ps = pj_ps.tile([128, C], F32, tag="pj", name="ps_v")
                for c in range(CT):
                    nc.tensor.matmul(
                        ps,
                        s["xt"][:, c, 256 * s["par"] + 128 * m:
                                256 * s["par"] + 128 * (m + 1)],
                        wv_sb[:, c, :],
                        start=(c == 0), stop=(c == CT - 1),
                    )
                nc.vector.tensor_tensor(out=v[:, m, :], in0=ps, in1=bvb_sb, op=ADD)

        def sc_batch(w, b):
            """Scores + exp for heads 2b, 2b+1.  The score matmuls run at
            raised scheduler priority: the exp chain paces the whole window,
            so PE should pick them the instant their psum bank frees."""
            s = st[w]
            sc = sc_ps.tile([128, 1024], F32, tag="sc", name="sc_t")
            a = apool.tile([128, 1024], BF16, name="a_sb")
            prio = tc.high_priority(offset=54 if b < 2 else 47)
            prio.__enter__()
            par = 256 * s["par"]
            for h2 in range(2):
                h = 2 * b + h2
                jq, base = h // 4, 32 * (h % 4)
                for t in range(TT):
                    # S^T[k-tile t, all q] = k_h[t-tile] @ q_h^T
                    nc.tensor.matmul(
                        sc[:, 512 * h2 + 256 * t: 512 * h2 + 256 * (t + 1)],
                        s["qk"][base:base + 32, CT + jq,
                                par + 128 * t:par + 128 * (t + 1)],
                        s["qk"][base:base + 32, jq, par:par + N],
                        start=True, stop=True,
                        tile_position=(base, 0),
                    )
            nc.scalar.activation(out=a, in_=sc, func=EXP, scale=SCALE)
            prio.__exit__(None, None, None)
            s.setdefault("a", {})[b] = a

        def av_batch(w, b):
            """attn @ v (token-major) + denominators for heads 2b, 2b+1."""
            s = st[w]
            if "o" not in s:
                s["o"] = [
                    o_ps.tile([128, 512], F32, tag="o", name="o_t") for _ in range(TT)
                ]
            a = s["a"].pop(b)
            for h2 in range(2):
                h = 2 * b + h2
                for qt in range(TT):
                    ot = s["o"][qt]
                    for t in range(TT):
                        lhsT = a[:, 512 * h2 + 256 * t + 128 * qt:
                                 512 * h2 + 256 * t + 128 * (qt + 1)]
                        nc.tensor.matmul(
                            ot[:, 32 * h:32 * (h + 1)],
                            lhsT, s["v"][:, t, 32 * h:32 * (h + 1)],
                            start=(t == 0), stop=(t == TT - 1),
                            skip_group_check=True,
                        )
                    for t in range(TT):
                        lhsT = a[:, 512 * h2 + 256 * t + 128 * qt:
                                 512 * h2 + 256 * t + 128 * (qt + 1)]
                        nc.tensor.matmul(
                            ot[:, C + h:C + h + 1],
                            lhsT, ones_sb[:, 0:1],
                            start=(t == 0), stop=(t == TT - 1),
                            skip_group_check=True,
                        )

        def norm_transpose(w, fast=False):
            """fast=True (pipeline drain): split normalize across DVE+GpSimd
            and transpose on the idle PE instead of the long-latency DMA
            XBAR path."""
            s = st[w]
            raws, recips = [], []
            for qt in range(TT):
                r = rpool.tile([128, H], F32, name="recip_sb")
                nc.vector.reciprocal_approx_fast(r, s["o"][qt][:, C:C + H])
                recips.append(r)
                raw = opool.tile([128, C], BF16, name="oraw_sb")
                if fast:  # halves, so normalize starts on the first half
                    nc.vector.tensor_copy(out=raw[:, 0:192], in_=s["o"][qt][:, 0:192])
                    nc.vector.tensor_copy(out=raw[:, 192:C], in_=s["o"][qt][:, 192:C])
                else:
                    nc.vector.tensor_copy(out=raw, in_=s["o"][qt][:, 0:C])
                raws.append(raw)
            del s["o"]
            oT = otpool.tile([128, CT, N], BF16, name="oT_sb")
            for qt in range(TT):
                onrm = o2pool.tile([128, C], BF16, name="onrm_sb")
                for h in range(H):
                    eng = nc.gpsimd if (not fast or h % 3 == 0) else nc.vector
                    eng.tensor_scalar(
                        out=onrm[:, 32 * h:32 * (h + 1)],
                        in0=raws[qt][:, 32 * h:32 * (h + 1)],
                        scalar1=recips[qt][:, h:h + 1], scalar2=None, op0=MULT,
                    )
                if fast:
                    tr = pj_ps.tile([128, CT, 128], BF16, tag="pj", name="tr_ps")
                    for g in range(CT):
                        nc.tensor.transpose(
                            tr[:, g, :], onrm[:, 128 * g:128 * (g + 1)], ident_sb
                        )
                    nc.vector.tensor_copy(
                        out=oT[:, :, 128 * qt:128 * (qt + 1)], in_=tr
                    )
                else:
                    nc.sync.dma_start_transpose(
                        out=oT[:, :, 128 * qt:128 * (qt + 1)], in_=onrm,
                    )
            s["oT"] = oT

        def fin_qt(w, qt):
            s = st[w]
            if "of" not in s:
                s["of"] = fpool.tile([128, TT, C], F32, name="of_sb")
            ps = pj_ps.tile([128, C], F32, tag="pj", name="ps_fin")
            for g in range(CT):
                nc.tensor.matmul(
                    ps,
                    s["oT"][:, g, 128 * qt:128 * (qt + 1)],
                    wo_sb[:, g, :],
                    start=(g == 0), stop=(g == CT - 1),
                )
            nc.vector.tensor_copy(out=s["of"][:, qt, :], in_=ps)

        def out_dma(w, split=False):
            s = st.pop(w)
            if split:  # drain: ship each q-tile independently
                for m in range(TT):
                    nc.sync.dma_start(
                        out=out_h.ap()[w, m].rearrange("p c -> p c"),
                        in_=s["of"][:, m, :],
                    )
            else:
                nc.sync.dma_start(
                    out=out_h.ap()[w].rearrange("m p c -> p m c"), in_=s["of"]
                )

        loop_ctx = tc.For_i(0, reps) if reps else None
        if loop_ctx is not None:
            ctx.enter_context(loop_ctx)
            dma_x(0)  # body-local x(0) load for the hardware-loop timing mode

        for si in range(wpc + 4):
            # wp: proj window, wa: attention window (batches 0-3; its batches
            # 4-5 + normalize run at the START of the next slot, after their
            # exps have finished), wo: output window.
            wp, wa, wn, wo = si, si - 1, si - 2, si - 3
            vp = wp < wpc
            va = 0 <= wa < wpc
            vn = 0 <= wn < wpc
            vo = 0 <= wo < wpc
            if vp and wp + 1 < wpc and (wp + 1) % 2 == 0:
                dma_x((wp + 1) // 2)
            fast_n = vn and wn >= wpc - 3
            if vn:
                av_batch(wn, 3)
                av_batch(wn, 4)
                av_batch(wn, 5)
            if vp and wp % 2 == 0:
                proj_qk_chunk(wp, 0)
            if va:
                sc_batch(wa, 2)
            if vn and not fast_n:
                # after pj0 so DVE's qk evictions aren't stuck behind the
                # norm chain's recip + raw-o evictions
                norm_transpose(wn)
            if vp and wp % 2 == 0:
                proj_qk_chunk(wp, 1)
            if va:
                av_batch(wa, 0)
                sc_batch(wa, 3)
            if vo:
                fin_qt(wo, 0)
            if va:
                sc_batch(wa, 4)
                sc_batch(wa, 5)
            if vp and wp % 2 == 0:
                proj_qk_chunk(wp, 2)
            if va:
                av_batch(wa, 1)
            if vo:
                fin_qt(wo, 1)
                out_dma(wo, split=(wo >= wpc - 2))
            if va:
                av_batch(wa, 2)
            if vp:
                proj_v(wp)
                # scores batches 0-1 of the just-projected window: emitted at
                # slot end so the Act exp chain for it starts ~1us earlier
                sc_batch(wp, 0)
                sc_batch(wp, 1)
            if fast_n:
                # drain windows: emit after fin/out work so the PE
                # transposes don't block ready output matmuls behind them
                norm_transpose(wn, fast=True)


    nc.compile()
    return nc


_PROGRAM = None


def _get_program():
    global _PROGRAM
    if _PROGRAM is None:
        _PROGRAM = build_program()
    return _PROGRAM


def make_in_maps(x, in_proj_weight, in_proj_bias, out_proj_weight):
    bf16 = ml_dtypes.bfloat16
    x = np.asarray(x, dtype=np.float32)
    in_proj_weight = np.asarray(in_proj_weight, dtype=np.float32)
    in_proj_bias = np.asarray(in_proj_bias, dtype=np.float32)
    out_proj_weight = np.asarray(out_proj_weight, dtype=np.float32)

    W = B * NWIN
    xt = x.reshape(W, N, C).transpose(0, 2, 1)          # [W, C, N]
    # window pairs: [W/2, 128, CT, 2, N] -> [W/2, 128, CT, 2N]
    xt = xt.reshape(W // 2, 2, CT, 128, N).transpose(0, 3, 2, 1, 4)
    xt = np.ascontiguousarray(xt).astype(bf16)
    xt = xt.reshape(NCORES, WPC // 2, 128, CT, 2 * N)

    def chanmajor(wT):  # [C, O] -> [128, CT, O]
        return np.ascontiguousarray(
            wT.reshape(CT, 128, -1).transpose(1, 0, 2)
        ).astype(bf16)

    wqkt = chanmajor(in_proj_weight[:2 * C].T)
    wvt = chanmajor(in_proj_weight[2 * C:].T)
    wot = chanmajor(out_proj_weight.T)
    bqkt = np.ascontiguousarray(in_proj_bias[:2 * C].reshape(2 * CT, 128).T)
    bvb = np.ascontiguousarray(np.broadcast_to(in_proj_bias[2 * C:], (128, C)))
    return [
        {"xt": xt[i], "wqkt": wqkt, "wvt": wvt, "wot": wot,
         "bqkt": bqkt, "bvb": bvb}
        for i in range(NCORES)
    ]


def assemble_out(results):
    outs = [r["out"].reshape(WPC, N, C) for r in results]
    return np.concatenate(outs).reshape(B, NWIN, N, C).astype(np.float32)


def kernel(x, in_proj_weight, in_proj_bias, out_proj_weight):
    nc = _get_program()
    in_maps = make_in_maps(x, in_proj_weight, in_proj_bias, out_proj_weight)
    res = run_bass_kernel_spmd(nc, in_maps, core_ids=list(range(NCORES)))
    return assemble_out(res.results)


# revision 87
# speedup vs baseline: 1.0056x; 1.0056x over previous
"""Windowed multi-head attention (DWAttention) Bass kernel for Trainium2.

Problem: x[B=2, n=64, N=256, C=384] -> per-window MHA (H=12, d=32) with fused
QKV projection + out_proj (no bias on out_proj, in_proj bias provided).

Strategy (8 NeuronCores, data-parallel over the B*n = 128 independent
windows -> 16 windows per core).

Cost-model facts driving the design (TRN2):
  * matmul time = out-free-size x cycles/row, independent of K and M.
  * bf16 runs 1 cycle/row always (fp32r needs free>=256 for full rate).
  * GpSimd (Pool) engine cannot touch PSUM; DVE/Act PSUM access costs a
    fixed ~125/185ns per instruction.
  * Act table load is 1.3us -> scalar engine runs ONLY Exp.
  * DMA XBAR transpose (16x128 tiles, 14ns/tile) does sbuf->sbuf bf16
    transposes for free (no PE/PSUM involvement).

Per window w (tokens N=256, channels C=384, 3 chan-tiles of 128):
  1. qk^T = W_qk @ x^T: 6 chan-major psum tiles [128,256] -> SBUF bf16 with
     per-partition bias add (DVE).  18 mms, free=256.
  2. v = x @ W_v^T: 2 token-major psum tiles [128,384] -> SBUF bf16 with
     broadcast bias add (DVE).  6 mms, free=384.
  3. Scores per head h: S^T[k,q] = k_h @ q_h^T (K=d=32): 24 mms free=256
     into [128,1024] psum (2 heads / 2 banks); exp via one Act op per
     2-head batch (scale=1/sqrt(d) fused) -> A^T bf16 in SBUF.
  4. attn@v token-major: out[q-tile, 32] per (head, q-tile, k-tile) with
     lhsT = A^T slice, rhs = v slice: 48 mms free=32; denominator via
     rhs=ones [128,1]: 48 mms free=1.  All accumulate in 2 psum banks
     [128, 512] = [12*32 o-cols | 12 den-cols | pad].
  5. recip(den) on DVE, raw o eviction psum->sbuf bf16 (DVE), per-head
     normalize SBUF->SBUF on GpSimd (24 tensor_scalar ops), then DMA XBAR
     transpose [128q,384c] -> oT [128c, 3, 128q] per q-tile.
  6. out = o @ W_o^T from oT: 6 mms free=384 -> psum, DVE evict, DMA out.

The 16 windows are software-pipelined: slot s emits proj(w=s),
attention(w=s-1), output(w=s-3) interleaved so PE never starves and PSUM
fits exactly in 8 banks: scores 2x2 + attn-out 2x1 + proj/fin 2x1.
"""

import numpy as np
from contextlib import ExitStack

import ml_dtypes

import concourse.bass as bass
import concourse.masks as masks
import concourse.mybir as mybir
import concourse.tile as tile
from concourse import bacc
from concourse.bass_utils import run_bass_kernel_spmd

# Problem constants (hardcoded per contract).
B, NWIN, N, C = 2, 64, 256, 384
H, D = 12, 32
SCALE = float(D) ** -0.5
NCORES = 8
WPC = (B * NWIN) // NCORES  # windows per core = 16
CT = C // 128               # channel tiles = 3
TT = N // 128               # token (q/k) tiles = 2
NB = H // 2                 # exp batches of 2 heads = 6

F32 = mybir.dt.float32
BF16 = mybir.dt.bfloat16
ADD = mybir.AluOpType.add
MULT = mybir.AluOpType.mult
EXP = mybir.ActivationFunctionType.Exp


def build_program(wpc=WPC, reps=0):
    """reps>0 wraps the whole per-core body in a hardware loop executing it
    reps times - used only for wall-clock HW timing (outputs unchanged)."""
    nc = bacc.Bacc()

    xt_h = nc.dram_tensor("xt", [wpc // 2, 128, CT, 2 * N], BF16, kind="ExternalInput")
    wqk_h = nc.dram_tensor("wqkt", [128, CT, 2 * C], BF16, kind="ExternalInput")
    wv_h = nc.dram_tensor("wvt", [128, CT, C], BF16, kind="ExternalInput")
    wo_h = nc.dram_tensor("wot", [128, CT, C], BF16, kind="ExternalInput")
    bqk_h = nc.dram_tensor("bqkt", [128, 2 * CT], F32, kind="ExternalInput")
    bvb_h = nc.dram_tensor("bvb", [128, C], F32, kind="ExternalInput")
    out_h = nc.dram_tensor("out", [wpc, TT, 128, C], F32, kind="ExternalOutput")

    with ExitStack() as ctx:
        tc = ctx.enter_context(tile.TileContext(nc))
        wpool = ctx.enter_context(tc.tile_pool(name="wpool", bufs=1))
        xpool = ctx.enter_context(tc.tile_pool(name="xpool", bufs=4))
        qkpool = ctx.enter_context(tc.tile_pool(name="qkpool", bufs=3))
        vpool = ctx.enter_context(tc.tile_pool(name="vpool", bufs=3))
        apool = ctx.enter_context(tc.tile_pool(name="apool", bufs=6))
        opool = ctx.enter_context(tc.tile_pool(name="opool", bufs=6))
        o2pool = ctx.enter_context(tc.tile_pool(name="o2pool", bufs=6))
        otpool = ctx.enter_context(tc.tile_pool(name="otpool", bufs=4))
        fpool = ctx.enter_context(tc.tile_pool(name="fpool", bufs=4))
        rpool = ctx.enter_context(tc.tile_pool(name="rpool", bufs=6))
        # PSUM: scores 2bufs x 2banks + attn-out 2bufs x 1bank
        #       + proj/fin 2bufs x 1bank = 8 banks exactly.
        sc_ps = ctx.enter_context(tc.tile_pool(name="sc_ps", bufs=2, space="PSUM"))
        o_ps = ctx.enter_context(tc.tile_pool(name="o_ps", bufs=2, space="PSUM"))
        pj_ps = ctx.enter_context(tc.tile_pool(name="pj_ps", bufs=2, space="PSUM"))

        st = {}  # per-window pipeline state

        def dma_x(p):
            """Load x^T for window pair (2p, 2p+1) in one tile: the QK
            projection batches both windows into free=512 matmuls."""
            xt = xpool.tile([128, CT, 2 * N], BF16, name="xt_sb")
            nc.sync.dma_start(out=xt, in_=xt_h.ap()[p])
            st[2 * p] = {"xt": xt, "par": 0}
            st[2 * p + 1] = {"xt": xt, "par": 1}

        # ---- one-time constants (ordered so the first proj's inputs land
        # first: x0 + wqk per c-tile + qk bias, then the rest; weight DMAs
        # ride the Activation engine's DGE queue, in parallel with SP's) ----
        xt0 = xpool.tile([128, CT, 2 * N], BF16, name="xt_sb")
        wqk_sb = wpool.tile([128, CT, 2 * C], BF16, name="wqk_sb")
        for c in range(CT):
            nc.sync.dma_start(out=wqk_sb[:, c, :], in_=wqk_h.ap()[:, c, :])
            nc.sync.dma_start(out=xt0[:, c, :], in_=xt_h.ap()[0, :, c, :])
        st[0] = {"xt": xt0, "par": 0}
        st[1] = {"xt": xt0, "par": 1}
        bqk_sb = wpool.tile([128, 2 * CT], F32, name="bqk_sb")
        nc.sync.dma_start(out=bqk_sb, in_=bqk_h.ap())
        wv_sb = wpool.tile([128, CT, C], BF16, name="wv_sb")
        nc.sync.dma_start(out=wv_sb, in_=wv_h.ap())
        bvb_sb = wpool.tile([128, C], F32, name="bvb_sb")
        nc.sync.dma_start(out=bvb_sb, in_=bvb_h.ap())
        wo_sb = wpool.tile([128, CT, C], BF16, name="wo_sb")
        nc.sync.dma_start(out=wo_sb, in_=wo_h.ap())
        ones_sb = wpool.tile([128, 1], BF16, name="ones_sb")
        nc.vector.memset(ones_sb, 1.0)
        ident_sb = wpool.tile([128, 128], BF16, name="ident_sb")
        masks.make_identity(nc, ident_sb)

        def proj_qk_chunk(w, j0):
            """Project q/k chan-tiles (j0, j0+CT) for windows (w, w+1) at
            once: the rhs spans the x^T pair tile, free=512 per matmul."""
            s = st[w]
            if "qk" not in s:
                s["qk"] = qkpool.tile([128, 2 * CT, 2 * N], BF16, name="qk_sb")
                st[w + 1]["qk"] = s["qk"]
            for j in (j0, j0 + CT):
                ps = pj_ps.tile([128, 2 * N], F32, tag="pj", name="ps_qk")
                for c in range(CT):
                    nc.tensor.matmul(
                        ps,
                        wqk_sb[:, c, 128 * j:128 * (j + 1)],
                        s["xt"][:, c, :],
                        start=(c == 0), stop=(c == CT - 1),
                    )
                nc.vector.tensor_scalar(
                    out=s["qk"][:, j, :], in0=ps,
                    scalar1=bqk_sb[:, j:j + 1], scalar2=None, op0=ADD,
                )

        def proj_v(w):
            s = st[w]
            v = vpool.tile([128, TT, C], BF16, name="v_sb")
            s["v"] = v
            for m in range(TT):
                ps = pj_ps.tile([128, C], F32, tag="pj", name="ps_v")
                for c in range(CT):
                    nc.tensor.matmul(
                        ps,
                        s["xt"][:, c, 256 * s["par"] + 128 * m:
                                256 * s["par"] + 128 * (m + 1)],
                        wv_sb[:, c, :],
                        start=(c == 0), stop=(c == CT - 1),
                    )
                nc.vector.tensor_tensor(out=v[:, m, :], in0=ps, in1=bvb_sb, op=ADD)

        def sc_batch(w, b):
            """Scores + exp for heads 2b, 2b+1.  The score matmuls run at
            raised scheduler priority: the exp chain paces the whole window,
            so PE should pick them the instant their psum bank frees."""
            s = st[w]
            sc = sc_ps.tile([128, 1024], F32, tag="sc", name="sc_t")
            a = apool.tile([128, 1024], BF16, name="a_sb")
            prio = tc.high_priority(offset=54 if b < 2 else 47)
            prio.__enter__()
            par = 256 * s["par"]
            for h2 in range(2):
                h = 2 * b + h2
                jq, base = h // 4, 32 * (h % 4)
                for t in range(TT):
                    # S^T[k-tile t, all q] = k_h[t-tile] @ q_h^T
                    nc.tensor.matmul(
                        sc[:, 512 * h2 + 256 * t: 512 * h2 + 256 * (t + 1)],
                        s["qk"][base:base + 32, CT + jq,
                                par + 128 * t:par + 128 * (t + 1)],
                        s["qk"][base:base + 32, jq, par:par + N],
                        start=True, stop=True,
                        tile_position=(base, 0),
                    )
            nc.scalar.activation(out=a, in_=sc, func=EXP, scale=SCALE)
            prio.__exit__(None, None, None)
            s.setdefault("a", {})[b] = a

        def av_batch(w, b):
            """attn @ v (token-major) + denominators for heads 2b, 2b+1."""
            s = st[w]
            if "o" not in s:
                s["o"] = [
                    o_ps.tile([128, 512], F32, tag="o", name="o_t") for _ in range(TT)
                ]
            a = s["a"].pop(b)
            for h2 in range(2):
                h = 2 * b + h2
                for qt in range(TT):
                    ot = s["o"][qt]
                    for t in range(TT):
                        lhsT = a[:, 512 * h2 + 256 * t + 128 * qt:
                                 512 * h2 + 256 * t + 128 * (qt + 1)]
                        nc.tensor.matmul(
                            ot[:, 32 * h:32 * (h + 1)],
                            lhsT, s["v"][:, t, 32 * h:32 * (h + 1)],
                            start=(t == 0), stop=(t == TT - 1),
                            skip_group_check=True,
                        )
                    for t in range(TT):
                        lhsT = a[:, 512 * h2 + 256 * t + 128 * qt:
                                 512 * h2 + 256 * t + 128 * (qt + 1)]
                        nc.tensor.matmul(
                            ot[:, C + h:C + h + 1],
                            lhsT, ones_sb[:, 0:1],
                            start=(t == 0), stop=(t == TT - 1),
                            skip_group_check=True,
                        )

        def norm_transpose(w, fast=False):
            """fast=True (pipeline drain): split normalize across DVE+GpSimd
            and transpose on the idle PE instead of the long-latency DMA
            XBAR path."""
            s = st[w]
            raws, recips = [], []
            for qt in range(TT):
                r = rpool.tile([128, H], F32, name="recip_sb")
                nc.vector.reciprocal_approx_fast(r, s["o"][qt][:, C:C + H])
                recips.append(r)
                raw = opool.tile([128, C], BF16, name="oraw_sb")
                if fast:  # halves, so normalize starts on the first half
                    nc.vector.tensor_copy(out=raw[:, 0:192], in_=s["o"][qt][:, 0:192])
                    nc.vector.tensor_copy(out=raw[:, 192:C], in_=s["o"][qt][:, 192:C])
                else:
                    nc.vector.tensor_copy(out=raw, in_=s["o"][qt][:, 0:C])
                raws.append(raw)
            del s["o"]
            oT = otpool.tile([128, CT, N], BF16, name="oT_sb")
            for qt in range(TT):
                onrm = o2pool.tile([128, C], BF16, name="onrm_sb")
                for h in range(H):
                    eng = nc.gpsimd if (not fast or h % 2 == 0) else nc.vector
                    eng.tensor_scalar(
                        out=onrm[:, 32 * h:32 * (h + 1)],
                        in0=raws[qt][:, 32 * h:32 * (h + 1)],
                        scalar1=recips[qt][:, h:h + 1], scalar2=None, op0=MULT,
                    )
                if fast:
                    tr = pj_ps.tile([128, CT, 128], BF16, tag="pj", name="tr_ps")
                    for g in range(CT):
                        nc.tensor.transpose(
                            tr[:, g, :], onrm[:, 128 * g:128 * (g + 1)], ident_sb
                        )
                    nc.vector.tensor_copy(
                        out=oT[:, :, 128 * qt:128 * (qt + 1)], in_=tr
                    )
                else:
                    nc.sync.dma_start_transpose(
                        out=oT[:, :, 128 * qt:128 * (qt + 1)], in_=onrm,
                    )
            s["oT"] = oT

        def fin_qt(w, qt):
            s = st[w]
            if "of" not in s:
                s["of"] = fpool.tile([128, TT, C], F32, name="of_sb")
            ps = pj_ps.tile([128, C], F32, tag="pj", name="ps_fin")
            for g in range(CT):
                nc.tensor.matmul(
                    ps,
                    s["oT"][:, g, 128 * qt:128 * (qt + 1)],
                    wo_sb[:, g, :],
                    start=(g == 0), stop=(g == CT - 1),
                )
            nc.vector.tensor_copy(out=s["of"][:, qt, :], in_=ps)

        def out_dma(w, split=False):
            s = st.pop(w)
            if split:  # drain: ship each q-tile independently
                for m in range(TT):
                    nc.sync.dma_start(
                        out=out_h.ap()[w, m].rearrange("p c -> p c"),
                        in_=s["of"][:, m, :],
                    )
            else:
                nc.sync.dma_start(
                    out=out_h.ap()[w].rearrange("m p c -> p m c"), in_=s["of"]
                )

        loop_ctx = tc.For_i(0, reps) if reps else None
        if loop_ctx is not None:
            ctx.enter_context(loop_ctx)
            dma_x(0)  # body-local x(0) load for the hardware-loop timing mode

        for si in range(wpc + 4):
            # wp: proj window, wa: attention window (batches 0-3; its batches
            # 4-5 + normalize run at the START of the next slot, after their
            # exps have finished), wo: output window.
            wp, wa, wn, wo = si, si - 1, si - 2, si - 3
            vp = wp < wpc
            va = 0 <= wa < wpc
            vn = 0 <= wn < wpc
            vo = 0 <= wo < wpc
            if vp and wp + 1 < wpc and (wp + 1) % 2 == 0:
                dma_x((wp + 1) // 2)
            fast_n = vn and wn >= wpc - 3
            if vn:
                av_batch(wn, 3)
                av_batch(wn, 4)
                av_batch(wn, 5)
            if vp and wp % 2 == 0:
                proj_qk_chunk(wp, 0)
            if va:
                sc_batch(wa, 2)
            if vn and not fast_n:
                # after pj0 so DVE's qk evictions aren't stuck behind the
                # norm chain's recip + raw-o evictions
                norm_transpose(wn)
            if vp and wp % 2 == 0:
                proj_qk_chunk(wp, 1)
            if va:
                av_batch(wa, 0)
                sc_batch(wa, 3)
            if vo:
                fin_qt(wo, 0)
            if va:
                sc_batch(wa, 4)
                sc_batch(wa, 5)
            if vp and wp % 2 == 0:
                proj_qk_chunk(wp, 2)
            if va:
                av_batch(wa, 1)
            if vo:
                fin_qt(wo, 1)
                out_dma(wo, split=(wo >= wpc - 2))
            if va:
                av_batch(wa, 2)
            if vp:
                proj_v(wp)
                # scores batches 0-1 of the just-projected window: emitted at
                # slot end so the Act exp chain for it starts ~1us earlier
                sc_batch(wp, 0)
                sc_batch(wp, 1)
            if fast_n:
                # drain windows: emit after fin/out work so the PE
                # transposes don't block ready output matmuls behind them
                norm_transpose(wn, fast=True)


    nc.compile()
    return nc


_PROGRAM = None


def _get_program():
    global _PROGRAM
    if _PROGRAM is None:
        _PROGRAM = build_program()
    return _PROGRAM


def make_in_maps(x, in_proj_weight, in_proj_bias, out_proj_weight):
    bf16 = ml_dtypes.bfloat16
    x = np.asarray(x, dtype=np.float32)
    in_proj_weight = np.asarray(in_proj_weight, dtype=np.float32)
    in_proj_bias = np.asarray(in_proj_bias, dtype=np.float32)
    out_proj_weight = np.asarray(out_proj_weight, dtype=np.float32)

    W = B * NWIN
    xt = x.reshape(W, N, C).transpose(0, 2, 1)          # [W, C, N]
    # window pairs: [W/2, 128, CT, 2, N] -> [W/2, 128, CT, 2N]
    xt = xt.reshape(W // 2, 2, CT, 128, N).transpose(0, 3, 2, 1, 4)
    xt = np.ascontiguousarray(xt).astype(bf16)
    xt = xt.reshape(NCORES, WPC // 2, 128, CT, 2 * N)

    def chanmajor(wT):  # [C, O] -> [128, CT, O]
        return np.ascontiguousarray(
            wT.reshape(CT, 128, -1).transpose(1, 0, 2)
        ).astype(bf16)

    wqkt = chanmajor(in_proj_weight[:2 * C].T)
    wvt = chanmajor(in_proj_weight[2 * C:].T)
    wot = chanmajor(out_proj_weight.T)
    bqkt = np.ascontiguousarray(in_proj_bias[:2 * C].reshape(2 * CT, 128).T)
    bvb = np.ascontiguousarray(np.broadcast_to(in_proj_bias[2 * C:], (128, C)))
    return [
        {"xt": xt[i], "wqkt": wqkt, "wvt": wvt, "wot": wot,
         "bqkt": bqkt, "bvb": bvb}
        for i in range(NCORES)
    ]


def assemble_out(results):
    outs = [r["out"].reshape(WPC, N, C) for r in results]
    return np.concatenate(outs).reshape(B, NWIN, N, C).astype(np.float32)


def kernel(x, in_proj_weight, in_proj_bias, out_proj_weight):
    nc = _get_program()
    in_maps = make_in_maps(x, in_proj_weight, in_proj_bias, out_proj_weight)
    res = run_bass_kernel_spmd(nc, in_maps, core_ids=list(range(NCORES)))
    return assemble_out(res.results)
